# revision 19
# baseline (speedup 1.0000x reference)
"""Trainium2 Bass kernel for 4-head spatial attention score softmax.

Reference computation:
    qk = einsum('bcxy,oc->boxy', fmap[1,256,64,64], W_qk[1024,256])
    q, k = split(qk, 2, axis=1)             # each [1, 512, 64, 64]
    q = q reshaped to heads, scaled by 128^-0.5
    sim[b,h,xy,uv] = q . k  (contraction over dim_head=128)
    out = softmax(sim, axis=-1)             # [1, 4, 4096, 4096] f32

Sharding: 8 cores = 4 heads x 2 query-halves. Each core projects q for its
2048 query columns + k for all 4096 columns, computes scores with fp16
matmuls, softmax, and streams a [2048, 4096] bf16 slab to HBM (host upcasts
to f32; bf16 rounding is ~0.1% rms vs the 2e-2 gate).

Per-core inputs are fp16 with the core's OWN query half as columns [0:2048]
(odd cores get the two 2048-column halves swapped) so the q projection uses
a static offset 0 and runs while the later fmap chunks still load. The host
un-swaps the output columns of odd cores.

softmax engine split per 128-query tile (4096 columns, 2 PSUM halves):
  - The host folds A = 128/ln2 into W_q, so PSUM scores arrive as A*s.
  - ScalarE: exp on cols [0:E] of each half via activation's free affine
    (scale = ln2/128 undoes A), bf16 out + accumulated row partial sums.
  - PE prefills the fast region [E:2048] of PSUM with the fast-exp magic
    constant B = 127*128 - C + 1.5*2^23 via a 3-partition matmul
    (B = 12582912 + 16256 - 7, each part bf16-exact); the score matmul
    accumulates A*s on top, so PSUM holds t = A*s + B directly.
  - DVE: ONE op per fast column: tensor_scalar reads the strided low-16
    bits of t (bitcast bf16 view of PSUM), which are exactly the bf16
    encoding of a linear-interp exp2 (~1.8% rms on these columns), writes
    the compacted bf16 row and accumulates row partial sums.
  - DVE: 4->1 partial-sum reduce, reciprocal, one 4096-wide bf16
    normalize multiply; DMA streams the row block out.

Hardware notes (from perfetto traces):
  - ACT exp measures ~1.19 ns/col + ~218ns ACTIVATION_READ_ACCUMULATOR per
    accum_out op; DVE 1x tensor_scalar ~ (250 + N)/0.96 ns.
  - The PE clock needs ~3us of dense matmul activity to reach 2.4 GHz and
    falls back to 1.2 GHz after idle gaps; warmup matmuls bridge the input
    load. DUMMY_MM zero-contribution matmuls (rhs=zeros, start=False) can
    be inserted into each half's accumulation group to keep PE streaming.
  - Back-to-back DMAs on one HWDGE queue interleave descriptors
    round-robin across the 16 DMA engines, so both input chunks would
    complete together; a tiny gating read serializes chunk 1 behind
    chunk 0 for the early projections.
  - A dummy activation at program start pulls the 1.3us Exp table load
    into ScalarE's idle input-load window.
  - PSUM: 8 banks x 512 f32; one matmul output must stay in one bank.
    Fast-region pieces are split at bank boundaries; matmul emission
    order (fast pieces first, ACT region last) keeps bank-sharing waits
    on ops the consumers need anyway.
"""

import numpy as np

import concourse.bacc as bacc
import concourse.mybir as mybir
import concourse.tile as tile
from concourse import bass_utils

HEADS = 4
DIM_HEAD = 128
C = 256          # input channels
XY = 4096        # 64*64 spatial positions
QCHUNK = 2048    # query positions per core
N_CORES = 8
SCALE = DIM_HEAD ** -0.5

F32 = mybir.dt.float32
BF16 = mybir.dt.bfloat16
F16 = mybir.dt.float16

# fast-exp constants: with scores pre-scaled by A = 128/ln2 (folded into
# W_q on host) and PSUM prefilled with B, the low 16 bits of the f32
# t = A*s + B are the bf16 encoding of ~exp(s) (linear-interp exp2,
# ~1.8% rms, tuned mean-unbiased by C_FE). The 1.5*2^23 magic addend
# forces round-to-integer inside the f32 mantissa.
FE_A = float(np.float32(128.0 / np.log(2.0)))
C_FE = 7.0
FE_B = float(np.float32(127.0 * 128.0 - C_FE + 12582912.0))
# B decomposed into bf16-exact INTEGER partials summed by a matmul of
# ones. Integers only: the f32 accumulation chain sits at ~1.26e7 where
# ulp=1, so fractional partials round away (costs ~+4% bias otherwise).
# Partition blocks (starts must be 0/32/64): 32x393216 + 32x504 + 1x121.
FE_B_PARTS = (393216.0, 504.0, 121.0)
assert 32 * (FE_B_PARTS[0] + FE_B_PARTS[1]) + FE_B_PARTS[2] == FE_B
LN2_128 = float(np.float32(np.log(2.0) / 128.0))  # ACT exp scale undoing A

# fast-exp columns per 2048-wide half. Tile serializes ACT-vs-DVE access
# to the SAME PSUM tile (any ranges), so the ACT region [0:E] and the DVE
# fast region live in SEPARATE PSUM tiles from two pools: psa (2 x 3
# banks = 1536 cols) and psd (2 x 1 bank = 512 cols). FQ=512 per half.
FQ = 512
E = 2048 - FQ


def _pieces(lo, hi):
    """Split [lo,hi) at 512-col PSUM bank boundaries."""
    out = []
    a = lo
    while a < hi:
        b = min(hi, (a // 512 + 1) * 512)
        out.append((a, b))
        a = b
    return out


def _emit(tc, fmap_k, wqkt, out):
    nc = tc.nc

    with tc.tile_pool(name="consts", bufs=1) as consts:
        # Weights transposed on host: [c, d] with c split into 2 partition
        # chunks. wqkt = [A*wq.T | wk.T] concatenated: one DMA instead of
        # two. A = 128/ln2 folded into wq on host.
        w_sb = consts.tile([128, 2, 2 * DIM_HEAD], F16)
        # fmap [256, n] -> [128p, 2, n]
        fk_sb = consts.tile([128, 2, XY], F16)
        warm_sb = consts.tile([128, 512], F16)
        junk = consts.tile([128, 4], F16)
        # warm init on GpSimd: it dispatches ~2us earlier than the vector
        # queue, so the PE-clock-ramping warmup matmuls start sooner.
        nc.gpsimd.memset(warm_sb, 0.0)
        fk_src = fmap_k.rearrange("(a p) n -> p a n", p=128)
        # Three DMA queues: the weights + fmap chunk 1 ride the GpSimd
        # SWDGE queue (no gate needed - separate ring), while chunk 0 goes
        # on the two HWDGE queues (one partition-group each) as two
        # column-subchunks. The gate read serializes subchunk 1 behind
        # subchunk 0 so the projection ladder starts on cols [0:1024]
        # ~2.5us earlier (back-to-back DMAs on one queue would otherwise
        # interleave descriptors and complete together).
        nc.gpsimd.dma_start(out=w_sb, in_=wqkt.rearrange("(a p) d -> p a d",
                                                         p=128))
        nc.sync.dma_start(out=fk_sb[:, 0, 0:1024], in_=fk_src[:, 0, 0:1024])
        nc.scalar.dma_start(out=fk_sb[:, 1, 0:1024], in_=fk_src[:, 1, 0:1024])
        nc.vector.memset(fk_sb[:, 0:2, 1024:1025], 0.0)  # init the gate cells
        nc.vector.tensor_copy(junk, fk_sb[:, 0:2, 1023:1025])
        nc.sync.dma_start(out=fk_sb[:, 0, 1024:2048],
                          in_=fk_src[:, 0, 1024:2048])
        nc.scalar.dma_start(out=fk_sb[:, 1, 1024:2048],
                            in_=fk_src[:, 1, 1024:2048])
        nc.gpsimd.dma_start(out=fk_sb[:, 0:2, 2048:XY],
                            in_=fk_src[:, 0:2, 2048:XY])

        q_sb = consts.tile([128, QCHUNK], F16)  # [d, x], pre-scaled by A
        k_sb = consts.tile([128, XY], F16)      # [d, uv]

        # B-prefill operands: ones lhsT [96, 128] and the B partials
        # replicated across 512 columns [96, 512], 32 partitions per part.
        # On GpSimd: keeps the DVE queue free during startup.
        ones_sb = consts.tile([96, 128], BF16)
        b_sb = consts.tile([96, 512], BF16)
        nc.gpsimd.memset(ones_sb, 1.0)
        nc.gpsimd.memset(b_sb[0:32, :], FE_B_PARTS[0])
        nc.gpsimd.memset(b_sb[32:64, :], FE_B_PARTS[1])
        nc.gpsimd.memset(b_sb[64:96, :], 0.0)
        nc.gpsimd.memset(b_sb[64:65, :], FE_B_PARTS[2])

        # dummy activation right away: the 1.3us Exp ACT_TABLE_LOAD fires
        # here, in ScalarE's idle input-load window, not inside the first
        # real exp on the critical path.
        tbl = consts.tile([128, 1], BF16)
        nc.scalar.activation(out=tbl, in_=warm_sb[:, 0:1],
                             func=mybir.ActivationFunctionType.Exp)

        # Two PSUM pools sized to the engine split: psa 2 x [128,1536]
        # (banks 0-5), psd 2 x [128,512] (banks 6-7). Separate tiles per
        # engine region so ACT's exp and DVE's compact never touch the
        # same PSUM tile (Tile serializes same-tile cross-engine access).
        # Projections ride the same rings (1536 + 512 col pieces).
        with tc.tile_pool(name="psa", bufs=2, space="PSUM") as psa_pool, \
             tc.tile_pool(name="psd", bufs=2, space="PSUM") as psd_pool, \
             tc.tile_pool(name="soft", bufs=6) as soft_pool, \
             tc.tile_pool(name="soft2", bufs=6) as soft2_pool, \
             tc.tile_pool(name="small", bufs=10) as small_pool:
            # PE warmup: dummy matmuls with no load deps keep TensorE busy
            # through the input-DMA window, ramping the HAM clock (a PE
            # idle gap drops it back to 1.2 GHz).
            warm_ps = psd_pool.tile([128, 384], F32, tag="psd")
            for i in range(18):
                nc.tensor.matmul(warm_ps, lhsT=warm_sb[:, 0:128],
                                 rhs=warm_sb[:, 0:384], start=True, stop=True)

            # ---- k projection for one 2048-col chunk g: each 512-col
            # piece gets its OWN psd tile, so the matmuls are purely
            # input-gated (a shared tile would serialize each piece's
            # matmuls behind the previous piece's copy: Tile treats any
            # cross-engine access to one PSUM tile as conflicting).
            # copy engines: chunk 0 alternates S/V; chunk 1 goes all-V
            # (ScalarE is busy with the first tiles' exps by then).
            def emit_kproj(g):
                for j in range(4):
                    kp = psd_pool.tile([128, 512], F32, tag="psd",
                                       name=f"kp{g}_{j}")
                    ksl = slice(g * 2048 + j * 512, g * 2048 + (j + 1) * 512)
                    nc.tensor.matmul(kp,
                                     lhsT=w_sb[:, 0, DIM_HEAD:2 * DIM_HEAD],
                                     rhs=fk_sb[:, 0, ksl],
                                     start=True, stop=False)
                    nc.tensor.matmul(kp,
                                     lhsT=w_sb[:, 1, DIM_HEAD:2 * DIM_HEAD],
                                     rhs=fk_sb[:, 1, ksl],
                                     start=False, stop=True)
                    if g == 0 and j % 2 == 0:
                        nc.scalar.copy(k_sb[:, ksl], kp)
                    else:
                        nc.vector.tensor_copy(k_sb[:, ksl], kp)

            # ---- q projection, 1024 columns per call. cq=0 rides the
            # psd ring as two 512 pieces (fastest path to tile 0); cq=1
            # uses one psa slot (free mid-ramp) so it doesn't lengthen the
            # psd ladder that kproj(1) needs.
            def emit_qproj(cq):
                if cq == 0:
                    for j in range(2):
                        qp = psd_pool.tile([128, 512], F32, tag="psd",
                                           name=f"qp0_{j}")
                        osl = slice(j * 512, (j + 1) * 512)
                        nc.tensor.matmul(qp, lhsT=w_sb[:, 0, 0:DIM_HEAD],
                                         rhs=fk_sb[:, 0, osl],
                                         start=True, stop=False)
                        nc.tensor.matmul(qp, lhsT=w_sb[:, 1, 0:DIM_HEAD],
                                         rhs=fk_sb[:, 1, osl],
                                         start=False, stop=True)
                        if j == 0:
                            nc.scalar.copy(q_sb[:, osl], qp)
                        else:
                            nc.vector.tensor_copy(q_sb[:, osl], qp)
                    return
                qp = psa_pool.tile([128, 1024], F32, tag="psa", name="qp1")
                for j in range(2):
                    osl = slice(1024 + j * 512, 1024 + (j + 1) * 512)
                    tgt = qp[:, j * 512:(j + 1) * 512]
                    nc.tensor.matmul(tgt, lhsT=w_sb[:, 0, 0:DIM_HEAD],
                                     rhs=fk_sb[:, 0, osl],
                                     start=True, stop=False)
                    nc.tensor.matmul(tgt, lhsT=w_sb[:, 1, 0:DIM_HEAD],
                                     rhs=fk_sb[:, 1, osl],
                                     start=False, stop=True)
                nc.scalar.copy(q_sb[:, 1024:1536], qp[:, 0:512])
                nc.vector.tensor_copy(q_sb[:, 1536:2048], qp[:, 512:1024])

            # ---- per-tile pieces ----
            ets = {}
            pps = {}
            dens = {}

            def emit_half(qt, half):
                if qt not in ets:
                    ets[qt] = soft_pool.tile([128, XY], BF16, tag="et",
                                             name=f"et{qt}")
                    pps[qt] = small_pool.tile([128, 4], F32, tag="pp",
                                              name=f"pp{qt}")
                    dens[qt] = small_pool.tile([128, 1], F32, tag="den",
                                               name=f"den{qt}")
                et, pp = ets[qt], pps[qt]
                qsl = q_sb[:, qt * 128:(qt + 1) * 128]
                koff = half * 2048
                a_ps = psa_pool.tile([128, E], F32, tag="psa",
                                     name=f"a_t{qt}h{half}")
                d_ps = psd_pool.tile([128, FQ], F32, tag="psd",
                                     name=f"d_t{qt}h{half}")
                # ACT region: 3 bank-sized score matmuls.
                for a, b in _pieces(0, E):
                    nc.tensor.matmul(a_ps[:, a:b], lhsT=qsl,
                                     rhs=k_sb[:, koff + a:koff + b],
                                     start=True, stop=True)
                # Fast region: prefill B with a matmul of ones, then
                # accumulate the A*s scores on top so PSUM holds t = A*s+B.
                nc.tensor.matmul(d_ps, lhsT=ones_sb, rhs=b_sb[:, 0:FQ],
                                 start=True, stop=False)
                nc.tensor.matmul(d_ps, lhsT=qsl,
                                 rhs=k_sb[:, koff + E:koff + 2048],
                                 start=False, stop=True)
                # DVE fast-exp: single op reads the strided low-16-bit
                # lanes of t (bf16 bitcast view of PSUM) into the bf16
                # output row with the row partial sum for free.
                nc.vector.tensor_scalar(
                    out=et[:, koff + E:koff + 2048],
                    in0=d_ps.bitcast(BF16)[:, 0:2 * FQ:2],
                    scalar1=1.0, scalar2=None,
                    op0=mybir.AluOpType.mult, op1=mybir.AluOpType.add,
                    accum_out=pp[:, 2 * half + 1:2 * half + 2])
                # ScalarE exp on cols [0:E]; scale undoes the A folded
                # into W_q (activation's affine stage is free).
                nc.scalar.activation(
                    out=et[:, koff:koff + E],
                    in_=a_ps,
                    func=mybir.ActivationFunctionType.Exp,
                    scale=LN2_128,
                    accum_out=pp[:, 2 * half:2 * half + 1])

            def emit_norm_store(qt, nsplit=1):
                et, pp, den = ets[qt], pps[qt], dens[qt]
                # out-of-place normalize into a fresh tile; frees the
                # raw-exp tile for reuse sooner.
                et2 = soft2_pool.tile([128, XY], BF16, tag="et2",
                                      name=f"et2_{qt}")
                nc.vector.tensor_reduce(den, pp[:, 0:4],
                                        axis=mybir.AxisListType.X,
                                        op=mybir.AluOpType.add)
                nc.vector.reciprocal(den, den)
                w = XY // nsplit
                for i in range(nsplit):
                    sl = slice(i * w, (i + 1) * w)
                    nc.vector.tensor_scalar_mul(et2[:, sl], et[:, sl], den)
                    # drain tail: the last tiles' pieces go out on the
                    # scalar HWDGE queue, parallel to the sync queue's
                    # backlog (ACT has no exp work left by then).
                    eng = nc.scalar if (nsplit > 1 and i % 2 == 0) else nc.sync
                    eng.dma_start(out=out[qt * 128:(qt + 1) * 128, sl],
                                  in_=et2[:, sl])

            # ---- schedule. Ring parity (each pool alternates r0/r1 per
            # allocation) lines up so every allocation's slot predecessor
            # releases at-or-before its own data dependencies:
            #   psd: warm(r0) kpB0(r1) d00(r0) kpB1(r1) d01(r0) d10(r1)...
            #   psa: qp0(r0) kpA0(r1) a00(r0) qp1(r1) kpA1(r0) a01(r1)
            #        a10(r0) a11(r1) a20(r0) ...
            # Ramp: per-piece projection tiles make the chain purely
            # data-flow: q/k pieces land ~0.7us apart, tile 0's exp starts
            # right after k[1024:1536]'s copy. h10 rides on the psa ring's
            # second slot so EXP(h10) follows EXP(h00) seamlessly while
            # fmap chunk 1 + kproj(1) are still in flight.
            emit_qproj(0)        # q cols 0:1024 (tiles 0-7), fmap chunk 0
            emit_kproj(0)        # k cols 0:2048, fmap chunk 0
            emit_half(0, 0)      # scores vs k[0:2048]
            emit_half(1, 0)
            emit_qproj(1)        # q cols 1024:2048 (tiles 8-15), chunk 0
            emit_kproj(1)        # k cols 2048:4096, fmap chunk 1
            emit_half(0, 1)
            emit_norm_store(0)
            emit_half(1, 1)
            NT = QCHUNK // 128
            for qt in range(2, NT):
                emit_half(qt, 0)
                # tail tiles split 2-ways across both HWDGE queues so the
                # final MBs drain in parallel at good per-piece efficiency
                emit_norm_store(qt - 1, nsplit=2 if qt >= NT - 3 else 1)
                emit_half(qt, 1)
            emit_norm_store(NT - 1, nsplit=2)


def build_program():
    nc = bacc.Bacc("TRN2", target_bir_lowering=False, debug=False,
                   enable_asserts=False)
    fmap_k = nc.dram_tensor("fmap_k", [C, XY], F16, kind="ExternalInput").ap()
    wqkt = nc.dram_tensor("wqkt", [C, 2 * DIM_HEAD], F16,
                          kind="ExternalInput").ap()
    out = nc.dram_tensor("out", [QCHUNK, XY], BF16, kind="ExternalOutput").ap()

    with tile.TileContext(nc) as tc:
        _emit(tc, fmap_k, wqkt, out)
    nc.compile()
    return nc


_CACHE = {}


def _get_nc():
    if "nc" not in _CACHE:
        _CACHE["nc"] = build_program()
    return _CACHE["nc"]


def make_in_maps(fmap, W_qk):
    fm = np.asarray(fmap, dtype=np.float32).reshape(C, XY)
    # per-query-half column orders: own half first
    fm_h = [np.ascontiguousarray(fm.astype(np.float16)),
            np.ascontiguousarray(
                np.concatenate([fm[:, QCHUNK:], fm[:, :QCHUNK]],
                               axis=1).astype(np.float16))]
    W = np.asarray(W_qk, dtype=np.float64)
    in_maps = []
    for core in range(N_CORES):
        hd, qhalf = divmod(core, 2)
        # A = 128/ln2 folded into wq: PSUM scores arrive pre-scaled for
        # the fast-exp bit trick; ACT's exp undoes it with scale=ln2/128.
        wq = W[hd * DIM_HEAD:(hd + 1) * DIM_HEAD] * (SCALE * FE_A)
        wk = W[HEADS * DIM_HEAD + hd * DIM_HEAD:
               HEADS * DIM_HEAD + (hd + 1) * DIM_HEAD]
        in_maps.append({
            "fmap_k": fm_h[qhalf],
            "wqkt": np.ascontiguousarray(
                np.concatenate([wq.T, wk.T], axis=1).astype(np.float16)),
        })
    return in_maps


def assemble(per_core_outs):
    out = np.empty((HEADS, XY, XY), dtype=np.float32)
    for core in range(N_CORES):
        hd, qhalf = divmod(core, 2)
        slab = np.asarray(per_core_outs[core]).astype(np.float32)
        if qhalf == 1:
            # core's k columns were [2048:4096 | 0:2048]: un-swap
            slab = np.concatenate([slab[:, QCHUNK:], slab[:, :QCHUNK]], axis=1)
        out[hd, qhalf * QCHUNK:(qhalf + 1) * QCHUNK, :] = slab
    return out.reshape(1, HEADS, XY, XY)


def kernel(fmap, W_qk, trace=False):
    nc = _get_nc()
    in_maps = make_in_maps(fmap, W_qk)
    res = bass_utils.run_bass_kernel_spmd(
        nc, in_maps, core_ids=list(range(N_CORES)), trace=trace)
    out = assemble([res.results[c]["out"] for c in range(N_CORES)])
    if trace:
        kernel.last_exec_time_ns = res.exec_time_ns
        kernel.last_results = res
    return out


# revision 20
# speedup vs baseline: 1.0196x; 1.0196x over previous
"""Trainium2 Bass kernel for 4-head spatial attention score softmax.

Reference computation:
    qk = einsum('bcxy,oc->boxy', fmap[1,256,64,64], W_qk[1024,256])
    q, k = split(qk, 2, axis=1)             # each [1, 512, 64, 64]
    q = q reshaped to heads, scaled by 128^-0.5
    sim[b,h,xy,uv] = q . k  (contraction over dim_head=128)
    out = softmax(sim, axis=-1)             # [1, 4, 4096, 4096] f32

Sharding: 8 cores = 4 heads x 2 query-halves. Each core projects q for its
2048 query columns + k for all 4096 columns, computes scores with fp16
matmuls, softmax, and streams a [2048, 4096] bf16 slab to HBM (host upcasts
to f32; bf16 rounding is ~0.1% rms vs the 2e-2 gate).

Per-core inputs are fp16 with the core's OWN query half as columns [0:2048]
(odd cores get the two 2048-column halves swapped) so the q projection uses
a static offset 0 and runs while the later fmap chunks still load. The host
un-swaps the output columns of odd cores.

softmax engine split per 128-query tile (4096 columns, 2 PSUM halves):
  - The host folds A = 128/ln2 into W_q, so PSUM scores arrive as A*s.
  - ScalarE: exp on cols [0:E] of each half via activation's free affine
    (scale = ln2/128 undoes A), bf16 out + accumulated row partial sums.
  - PE prefills the fast region [E:2048] of PSUM with the fast-exp magic
    constant B = 127*128 - C + 1.5*2^23 via a 3-partition matmul
    (B = 12582912 + 16256 - 7, each part bf16-exact); the score matmul
    accumulates A*s on top, so PSUM holds t = A*s + B directly.
  - DVE: ONE op per fast column: tensor_scalar reads the strided low-16
    bits of t (bitcast bf16 view of PSUM), which are exactly the bf16
    encoding of a linear-interp exp2 (~1.8% rms on these columns), writes
    the compacted bf16 row and accumulates row partial sums.
  - DVE: 4->1 partial-sum reduce, reciprocal, one 4096-wide bf16
    normalize multiply; DMA streams the row block out.

Hardware notes (from perfetto traces):
  - ACT exp measures ~1.19 ns/col + ~218ns ACTIVATION_READ_ACCUMULATOR per
    accum_out op; DVE 1x tensor_scalar ~ (250 + N)/0.96 ns.
  - The PE clock needs ~3us of dense matmul activity to reach 2.4 GHz and
    falls back to 1.2 GHz after idle gaps; warmup matmuls bridge the input
    load. DUMMY_MM zero-contribution matmuls (rhs=zeros, start=False) can
    be inserted into each half's accumulation group to keep PE streaming.
  - Back-to-back DMAs on one HWDGE queue interleave descriptors
    round-robin across the 16 DMA engines, so both input chunks would
    complete together; a tiny gating read serializes chunk 1 behind
    chunk 0 for the early projections.
  - A dummy activation at program start pulls the 1.3us Exp table load
    into ScalarE's idle input-load window.
  - PSUM: 8 banks x 512 f32; one matmul output must stay in one bank.
    Fast-region pieces are split at bank boundaries; matmul emission
    order (fast pieces first, ACT region last) keeps bank-sharing waits
    on ops the consumers need anyway.
"""

import numpy as np

import concourse.bacc as bacc
import concourse.mybir as mybir
import concourse.tile as tile
from concourse import bass_utils

HEADS = 4
DIM_HEAD = 128
C = 256          # input channels
XY = 4096        # 64*64 spatial positions
QCHUNK = 2048    # query positions per core
N_CORES = 8
SCALE = DIM_HEAD ** -0.5

F32 = mybir.dt.float32
BF16 = mybir.dt.bfloat16
F16 = mybir.dt.float16

# fast-exp constants: with scores pre-scaled by A = 128/ln2 (folded into
# W_q on host) and PSUM prefilled with B, the low 16 bits of the f32
# t = A*s + B are the bf16 encoding of ~exp(s) (linear-interp exp2,
# ~1.8% rms, tuned mean-unbiased by C_FE). The 1.5*2^23 magic addend
# forces round-to-integer inside the f32 mantissa.
FE_A = float(np.float32(128.0 / np.log(2.0)))
C_FE = 7.0
FE_B = float(np.float32(127.0 * 128.0 - C_FE + 12582912.0))
# B decomposed into bf16-exact INTEGER partials summed by a matmul of
# ones. Integers only: the f32 accumulation chain sits at ~1.26e7 where
# ulp=1, so fractional partials round away (costs ~+4% bias otherwise).
# Partition blocks (starts must be 0/32/64): 32x393216 + 32x504 + 1x121.
FE_B_PARTS = (393216.0, 504.0, 121.0)
assert 32 * (FE_B_PARTS[0] + FE_B_PARTS[1]) + FE_B_PARTS[2] == FE_B
LN2_128 = float(np.float32(np.log(2.0) / 128.0))  # ACT exp scale undoing A

# fast-exp columns per 2048-wide half. Tile serializes ACT-vs-DVE access
# to the SAME PSUM tile (any ranges), so the ACT region [0:E] and the DVE
# fast region live in SEPARATE PSUM tiles from two pools: psa (2 x 3
# banks = 1536 cols) and psd (2 x 1 bank = 512 cols). FQ=512 per half.
FQ = 512
E = 2048 - FQ


def _pieces(lo, hi):
    """Split [lo,hi) at 512-col PSUM bank boundaries."""
    out = []
    a = lo
    while a < hi:
        b = min(hi, (a // 512 + 1) * 512)
        out.append((a, b))
        a = b
    return out


def _emit(tc, fmap_k, wqkt, out):
    nc = tc.nc

    with tc.tile_pool(name="consts", bufs=1) as consts:
        # Weights transposed on host: [c, d] with c split into 2 partition
        # chunks. wqkt = [A*wq.T | wk.T] concatenated: one DMA instead of
        # two. A = 128/ln2 folded into wq on host.
        w_sb = consts.tile([128, 2, 2 * DIM_HEAD], F16)
        # fmap [256, n] -> [128p, 2, n]
        fk_sb = consts.tile([128, 2, XY], F16)
        warm_sb = consts.tile([128, 512], F16)
        junk = consts.tile([128, 4], F16)
        # warm init on GpSimd: it dispatches ~2us earlier than the vector
        # queue, so the PE-clock-ramping warmup matmuls start sooner.
        nc.gpsimd.memset(warm_sb, 0.0)
        fk_src = fmap_k.rearrange("(a p) n -> p a n", p=128)
        # Weights ride the GpSimd SWDGE queue (frees the sync queue head
        # for fmap). fmap goes in two 2048-column chunks; each chunk's
        # partition-groups use the two HWDGE queues concurrently. Chunk 1
        # is gated behind a tiny read spanning both chunks' SBUF cells
        # (back-to-back DMAs on one queue would otherwise interleave
        # descriptors round-robin and complete together).
        nc.gpsimd.dma_start(out=w_sb, in_=wqkt.rearrange("(a p) d -> p a d",
                                                         p=128))
        nc.sync.dma_start(out=fk_sb[:, 0, 0:2048], in_=fk_src[:, 0, 0:2048])
        nc.scalar.dma_start(out=fk_sb[:, 1, 0:2048], in_=fk_src[:, 1, 0:2048])
        nc.vector.memset(fk_sb[:, 0:2, 2048:2049], 0.0)  # init the gate cells
        nc.vector.tensor_copy(junk, fk_sb[:, 0:2, 2047:2049])
        nc.sync.dma_start(out=fk_sb[:, 0, 2048:XY], in_=fk_src[:, 0, 2048:XY])
        nc.scalar.dma_start(out=fk_sb[:, 1, 2048:XY], in_=fk_src[:, 1, 2048:XY])

        q_sb = consts.tile([128, QCHUNK], F16)  # [d, x], pre-scaled by A
        k_sb = consts.tile([128, XY], F16)      # [d, uv]

        # B-prefill operands: ones lhsT [96, 128] and the B partials
        # replicated across 512 columns [96, 512], 32 partitions per part.
        # On GpSimd: keeps the DVE queue free during startup.
        ones_sb = consts.tile([96, 128], BF16)
        b_sb = consts.tile([96, 512], BF16)
        nc.gpsimd.memset(ones_sb, 1.0)
        nc.gpsimd.memset(b_sb[0:32, :], FE_B_PARTS[0])
        nc.gpsimd.memset(b_sb[32:64, :], FE_B_PARTS[1])
        nc.gpsimd.memset(b_sb[64:96, :], 0.0)
        nc.gpsimd.memset(b_sb[64:65, :], FE_B_PARTS[2])

        # dummy activation right away: the 1.3us Exp ACT_TABLE_LOAD fires
        # here, in ScalarE's idle input-load window, not inside the first
        # real exp on the critical path.
        tbl = consts.tile([128, 1], BF16)
        nc.scalar.activation(out=tbl, in_=warm_sb[:, 0:1],
                             func=mybir.ActivationFunctionType.Exp)

        # Two PSUM pools sized to the engine split: psa 2 x [128,1536]
        # (banks 0-5), psd 2 x [128,512] (banks 6-7). Separate tiles per
        # engine region so ACT's exp and DVE's compact never touch the
        # same PSUM tile (Tile serializes same-tile cross-engine access).
        # Projections ride the same rings (1536 + 512 col pieces).
        with tc.tile_pool(name="psa", bufs=2, space="PSUM") as psa_pool, \
             tc.tile_pool(name="psd", bufs=2, space="PSUM") as psd_pool, \
             tc.tile_pool(name="soft", bufs=6) as soft_pool, \
             tc.tile_pool(name="soft2", bufs=6) as soft2_pool, \
             tc.tile_pool(name="small", bufs=10) as small_pool:
            # PE warmup: dummy matmuls with no load deps keep TensorE busy
            # through the input-DMA window, ramping the HAM clock (a PE
            # idle gap drops it back to 1.2 GHz).
            warm_ps = psd_pool.tile([128, 384], F32, tag="psd")
            for i in range(18):
                nc.tensor.matmul(warm_ps, lhsT=warm_sb[:, 0:128],
                                 rhs=warm_sb[:, 0:384], start=True, stop=True)

            # ---- k projection for one 2048-col chunk g: each 512-col
            # piece gets its OWN psd tile, so the matmuls are purely
            # input-gated (a shared tile would serialize each piece's
            # matmuls behind the previous piece's copy: Tile treats any
            # cross-engine access to one PSUM tile as conflicting).
            # copy engines: chunk 0 alternates S/V; chunk 1 goes all-V
            # (ScalarE is busy with the first tiles' exps by then).
            def emit_kproj(g):
                for j in range(4):
                    kp = psd_pool.tile([128, 512], F32, tag="psd",
                                       name=f"kp{g}_{j}")
                    ksl = slice(g * 2048 + j * 512, g * 2048 + (j + 1) * 512)
                    nc.tensor.matmul(kp,
                                     lhsT=w_sb[:, 0, DIM_HEAD:2 * DIM_HEAD],
                                     rhs=fk_sb[:, 0, ksl],
                                     start=True, stop=False)
                    nc.tensor.matmul(kp,
                                     lhsT=w_sb[:, 1, DIM_HEAD:2 * DIM_HEAD],
                                     rhs=fk_sb[:, 1, ksl],
                                     start=False, stop=True)
                    if g == 0 and j % 2 == 0:
                        nc.scalar.copy(k_sb[:, ksl], kp)
                    else:
                        nc.vector.tensor_copy(k_sb[:, ksl], kp)

            # ---- q projection, 1024 columns per call. cq=0 rides the
            # psd ring as two 512 pieces (fastest path to tile 0); cq=1
            # uses one psa slot (free mid-ramp) so it doesn't lengthen the
            # psd ladder that kproj(1) needs.
            def emit_qproj(cq):
                if cq == 0:
                    for j in range(2):
                        qp = psd_pool.tile([128, 512], F32, tag="psd",
                                           name=f"qp0_{j}")
                        osl = slice(j * 512, (j + 1) * 512)
                        nc.tensor.matmul(qp, lhsT=w_sb[:, 0, 0:DIM_HEAD],
                                         rhs=fk_sb[:, 0, osl],
                                         start=True, stop=False)
                        nc.tensor.matmul(qp, lhsT=w_sb[:, 1, 0:DIM_HEAD],
                                         rhs=fk_sb[:, 1, osl],
                                         start=False, stop=True)
                        if j == 0:
                            nc.scalar.copy(q_sb[:, osl], qp)
                        else:
                            nc.vector.tensor_copy(q_sb[:, osl], qp)
                    return
                qp = psa_pool.tile([128, 1024], F32, tag="psa", name="qp1")
                for j in range(2):
                    osl = slice(1024 + j * 512, 1024 + (j + 1) * 512)
                    tgt = qp[:, j * 512:(j + 1) * 512]
                    nc.tensor.matmul(tgt, lhsT=w_sb[:, 0, 0:DIM_HEAD],
                                     rhs=fk_sb[:, 0, osl],
                                     start=True, stop=False)
                    nc.tensor.matmul(tgt, lhsT=w_sb[:, 1, 0:DIM_HEAD],
                                     rhs=fk_sb[:, 1, osl],
                                     start=False, stop=True)
                nc.scalar.copy(q_sb[:, 1024:1536], qp[:, 0:512])
                nc.vector.tensor_copy(q_sb[:, 1536:2048], qp[:, 512:1024])

            # ---- per-tile pieces ----
            ets = {}
            pps = {}
            dens = {}

            def emit_half(qt, half):
                if qt not in ets:
                    ets[qt] = soft_pool.tile([128, XY], BF16, tag="et",
                                             name=f"et{qt}")
                    pps[qt] = small_pool.tile([128, 4], F32, tag="pp",
                                              name=f"pp{qt}")
                    dens[qt] = small_pool.tile([128, 1], F32, tag="den",
                                               name=f"den{qt}")
                et, pp = ets[qt], pps[qt]
                qsl = q_sb[:, qt * 128:(qt + 1) * 128]
                koff = half * 2048
                a_ps = psa_pool.tile([128, E], F32, tag="psa",
                                     name=f"a_t{qt}h{half}")
                d_ps = psd_pool.tile([128, FQ], F32, tag="psd",
                                     name=f"d_t{qt}h{half}")
                # ACT region: 3 bank-sized score matmuls.
                for a, b in _pieces(0, E):
                    nc.tensor.matmul(a_ps[:, a:b], lhsT=qsl,
                                     rhs=k_sb[:, koff + a:koff + b],
                                     start=True, stop=True)
                # Fast region: prefill B with a matmul of ones, then
                # accumulate the A*s scores on top so PSUM holds t = A*s+B.
                nc.tensor.matmul(d_ps, lhsT=ones_sb, rhs=b_sb[:, 0:FQ],
                                 start=True, stop=False)
                nc.tensor.matmul(d_ps, lhsT=qsl,
                                 rhs=k_sb[:, koff + E:koff + 2048],
                                 start=False, stop=True)
                # DVE fast-exp: single op reads the strided low-16-bit
                # lanes of t (bf16 bitcast view of PSUM) into the bf16
                # output row with the row partial sum for free.
                nc.vector.tensor_scalar(
                    out=et[:, koff + E:koff + 2048],
                    in0=d_ps.bitcast(BF16)[:, 0:2 * FQ:2],
                    scalar1=1.0, scalar2=None,
                    op0=mybir.AluOpType.mult, op1=mybir.AluOpType.add,
                    accum_out=pp[:, 2 * half + 1:2 * half + 2])
                # ScalarE exp on cols [0:E]; scale undoes the A folded
                # into W_q (activation's affine stage is free).
                nc.scalar.activation(
                    out=et[:, koff:koff + E],
                    in_=a_ps,
                    func=mybir.ActivationFunctionType.Exp,
                    scale=LN2_128,
                    accum_out=pp[:, 2 * half:2 * half + 1])

            def emit_norm_store(qt, nsplit=1):
                et, pp, den = ets[qt], pps[qt], dens[qt]
                # out-of-place normalize into a fresh tile; frees the
                # raw-exp tile for reuse sooner.
                et2 = soft2_pool.tile([128, XY], BF16, tag="et2",
                                      name=f"et2_{qt}")
                nc.vector.tensor_reduce(den, pp[:, 0:4],
                                        axis=mybir.AxisListType.X,
                                        op=mybir.AluOpType.add)
                nc.vector.reciprocal(den, den)
                w = XY // nsplit
                for i in range(nsplit):
                    sl = slice(i * w, (i + 1) * w)
                    nc.vector.tensor_scalar_mul(et2[:, sl], et[:, sl], den)
                    # drain tail: the last tiles' pieces go out on the
                    # scalar HWDGE queue, parallel to the sync queue's
                    # backlog (ACT has no exp work left by then).
                    eng = nc.scalar if (nsplit > 1 and i % 2 == 0) else nc.sync
                    eng.dma_start(out=out[qt * 128:(qt + 1) * 128, sl],
                                  in_=et2[:, sl])

            # ---- schedule. Ring parity (each pool alternates r0/r1 per
            # allocation) lines up so every allocation's slot predecessor
            # releases at-or-before its own data dependencies:
            #   psd: warm(r0) kpB0(r1) d00(r0) kpB1(r1) d01(r0) d10(r1)...
            #   psa: qp0(r0) kpA0(r1) a00(r0) qp1(r1) kpA1(r0) a01(r1)
            #        a10(r0) a11(r1) a20(r0) ...
            # Ramp: per-piece projection tiles make the chain purely
            # data-flow: q/k pieces land ~0.7us apart, tile 0's exp starts
            # right after k[1024:1536]'s copy. h10 rides on the psa ring's
            # second slot so EXP(h10) follows EXP(h00) seamlessly while
            # fmap chunk 1 + kproj(1) are still in flight.
            emit_qproj(0)        # q cols 0:1024 (tiles 0-7), fmap chunk 0
            emit_kproj(0)        # k cols 0:2048, fmap chunk 0
            emit_half(0, 0)      # scores vs k[0:2048]
            emit_half(1, 0)
            emit_qproj(1)        # q cols 1024:2048 (tiles 8-15), chunk 0
            emit_kproj(1)        # k cols 2048:4096, fmap chunk 1
            emit_half(0, 1)
            emit_norm_store(0)
            emit_half(1, 1)
            NT = QCHUNK // 128
            for qt in range(2, NT):
                emit_half(qt, 0)
                # tail tiles split 2-ways across both HWDGE queues so the
                # final MBs drain in parallel at good per-piece efficiency
                emit_norm_store(qt - 1, nsplit=2 if qt >= NT - 3 else 1)
                emit_half(qt, 1)
            emit_norm_store(NT - 1, nsplit=2)


def build_program():
    nc = bacc.Bacc("TRN2", target_bir_lowering=False, debug=False,
                   enable_asserts=False)
    fmap_k = nc.dram_tensor("fmap_k", [C, XY], F16, kind="ExternalInput").ap()
    wqkt = nc.dram_tensor("wqkt", [C, 2 * DIM_HEAD], F16,
                          kind="ExternalInput").ap()
    out = nc.dram_tensor("out", [QCHUNK, XY], BF16, kind="ExternalOutput").ap()

    with tile.TileContext(nc) as tc:
        _emit(tc, fmap_k, wqkt, out)
    nc.compile()
    return nc


_CACHE = {}


def _get_nc():
    if "nc" not in _CACHE:
        _CACHE["nc"] = build_program()
    return _CACHE["nc"]


def make_in_maps(fmap, W_qk):
    fm = np.asarray(fmap, dtype=np.float32).reshape(C, XY)
    # per-query-half column orders: own half first
    fm_h = [np.ascontiguousarray(fm.astype(np.float16)),
            np.ascontiguousarray(
                np.concatenate([fm[:, QCHUNK:], fm[:, :QCHUNK]],
                               axis=1).astype(np.float16))]
    W = np.asarray(W_qk, dtype=np.float64)
    in_maps = []
    for core in range(N_CORES):
        hd, qhalf = divmod(core, 2)
        # A = 128/ln2 folded into wq: PSUM scores arrive pre-scaled for
        # the fast-exp bit trick; ACT's exp undoes it with scale=ln2/128.
        wq = W[hd * DIM_HEAD:(hd + 1) * DIM_HEAD] * (SCALE * FE_A)
        wk = W[HEADS * DIM_HEAD + hd * DIM_HEAD:
               HEADS * DIM_HEAD + (hd + 1) * DIM_HEAD]
        in_maps.append({
            "fmap_k": fm_h[qhalf],
            "wqkt": np.ascontiguousarray(
                np.concatenate([wq.T, wk.T], axis=1).astype(np.float16)),
        })
    return in_maps


def assemble(per_core_outs):
    out = np.empty((HEADS, XY, XY), dtype=np.float32)
    for core in range(N_CORES):
        hd, qhalf = divmod(core, 2)
        slab = np.asarray(per_core_outs[core]).astype(np.float32)
        if qhalf == 1:
            # core's k columns were [2048:4096 | 0:2048]: un-swap
            slab = np.concatenate([slab[:, QCHUNK:], slab[:, :QCHUNK]], axis=1)
        out[hd, qhalf * QCHUNK:(qhalf + 1) * QCHUNK, :] = slab
    return out.reshape(1, HEADS, XY, XY)


def kernel(fmap, W_qk, trace=False):
    nc = _get_nc()
    in_maps = make_in_maps(fmap, W_qk)
    res = bass_utils.run_bass_kernel_spmd(
        nc, in_maps, core_ids=list(range(N_CORES)), trace=trace)
    out = assemble([res.results[c]["out"] for c in range(N_CORES)])
    if trace:
        kernel.last_exec_time_ns = res.exec_time_ns
        kernel.last_results = res
    return out
